# revision 1
# baseline (speedup 1.0000x reference)
"""GCNConv (gather -> scale -> segment-sum -> linear) on 8 trn2 NeuronCores.

Math: out = segment_sum(h[src] * edge_w, dst), h = x @ W + b.
Using associativity, per 128-node destination block B:
    out[B] = (sum_e onehot(dst_e)^T (edge_w_e * x[src_e])) @ W + 1*b
           = S_B @ W + b,  with S_B^T accumulated on the tensor engine as
    ST[din, node] += x_chunk^T @ onehot_chunk   (PSUM accumulation)
so x rows are gathered once (512B rows via dma_gather), never projected
per-edge, and the scatter-add is a matmul against a weighted one-hot built
with a single fused DVE tensor_scalar (is_equal x mult) per 128-edge chunk.

Sharding: destination-owner. The 782 dst blocks are assigned to 8 cores
balanced by edge count; each core computes its blocks' outputs completely
(no collective). Indices for dma_gather are int16, so x is viewed as 4
sub-tables of <=32768 rows and edges are binned by (dst block, src>>15).
All loop bounds are derived from the data at call time and baked into one
SPMD program (per-core segment counts padded up to the max across cores).
"""

import sys

sys.path.insert(0, "/opt/trn_rl_repo")

from contextlib import ExitStack

import numpy as np

import concourse.bacc as bacc
import concourse.mybir as mybir
from concourse import tile
from concourse.bass_utils import run_bass_kernel_spmd

N_NODES = 100000
N_EDGES = 1600000
D_IN = 128
D_OUT = 64
N_CORES = 8
BLK = 128                      # dst nodes per block (= one-hot width)
SRC_CH = 1 << 15               # rows per gather sub-table (int16 idx limit)
N_SBIN = (N_NODES + SRC_CH - 1) // SRC_CH      # 4
N_BLOCKS = (N_NODES + BLK - 1) // BLK          # 782
HG = 3                         # dst blocks per "halfgroup" (PSUM-resident set)
POS = 99                       # positions per core (33 halfgroups of 3)
N_HG = POS // HG


def _plan(src, dst):
    """Derive the shared SPMD loop structure + per-core edge orderings."""
    block = dst >> 7
    sbin = src >> 15
    key = block * N_SBIN + sbin
    order = np.argsort(key, kind="stable")
    counts = np.bincount(key, minlength=N_BLOCKS * N_SBIN).reshape(N_BLOCKS, N_SBIN)
    seg_start = np.zeros(N_BLOCKS * N_SBIN + 1, np.int64)
    seg_start[1:] = np.cumsum(counts.ravel())

    # Balanced assignment of blocks to cores (greedy, largest first).
    blk_tot = counts.sum(1)
    pos_block = np.full((N_CORES, POS), -1, np.int64)
    core_tot = np.zeros(N_CORES, np.int64)
    core_n = np.zeros(N_CORES, np.int64)
    for b in np.argsort(-blk_tot, kind="stable"):
        ok = np.where(core_n < POS)[0]
        c = ok[np.argmin(core_tot[ok])]
        pos_block[c, core_n[c]] = b
        core_n[c] += 1
        core_tot[c] += blk_tot[b]
    # positions on each core sorted by size desc -> rank-matched across cores
    for c in range(N_CORES):
        nb = int(core_n[c])
        bl = pos_block[c, :nb]
        pos_block[c, :nb] = bl[np.argsort(-blk_tot[bl], kind="stable")]

    # seg sizes [core, pos, sbin]; chunks per (pos, sbin) = max over cores
    segcnt = np.where(
        pos_block[:, :, None] >= 0,
        counts[np.clip(pos_block, 0, None)],
        0,
    )                                             # [8, POS, 4]
    K = -(-segcnt.max(0) // BLK)                  # [POS, 4] chunks (ceil)
    return order, counts, seg_start, pos_block, segcnt, K


def _offsets(K):
    """Static stream layouts shared by all cores.

    gather order  : (hg, s, p in hg)  -> idx stream offsets + call sizes
    emission order: (hg, p in hg, s)  -> dst/ew stream chunk offsets
    """
    goff = np.zeros((POS, N_SBIN), np.int64)      # idx slot offset of (p, s)
    call_off = np.zeros((N_HG, N_SBIN), np.int64)
    call_n = np.zeros((N_HG, N_SBIN), np.int64)
    eoff = np.zeros((POS, N_SBIN), np.int64)      # chunk offset of (p, s)
    g = 0
    t = 0
    for hg in range(N_HG):
        ps = range(hg * HG, hg * HG + HG)
        for s in range(N_SBIN):
            call_off[hg, s] = g
            for p in ps:
                goff[p, s] = g
                g += int(K[p, s]) * BLK
            call_n[hg, s] = g - call_off[hg, s]
        for p in ps:
            for s in range(N_SBIN):
                eoff[p, s] = t
                t += int(K[p, s])
    return goff, call_off, call_n, eoff, int(g), int(t)


def _build(K, call_off, call_n, eoff, ntot, ttot, hg_limit=None,
           skip_gather=False, skip_compute=False):
    nc = bacc.Bacc("TRN2", target_bir_lowering=False, debug=False,
                   enable_asserts=False, num_devices=N_CORES,
                   num_swdge_queues=2)
    f32 = mybir.dt.float32
    x_d = nc.dram_tensor("x", [N_NODES, D_IN], f32, kind="ExternalInput")
    w_d = nc.dram_tensor("w", [D_IN, D_OUT], f32, kind="ExternalInput")
    bias_d = nc.dram_tensor("bias", [128, D_OUT], f32, kind="ExternalInput")
    iota_d = nc.dram_tensor("iota", [128, BLK], f32, kind="ExternalInput")
    idx_d = nc.dram_tensor("idx", [128, ntot // 16], mybir.dt.int16,
                           kind="ExternalInput")
    dstm_d = nc.dram_tensor("dstm", [128, ttot], f32, kind="ExternalInput")
    ewm_d = nc.dram_tensor("ewm", [128, ttot], f32, kind="ExternalInput")
    wsum_d = nc.dram_tensor("wsum", [128, POS], f32, kind="ExternalInput")
    out_d = nc.dram_tensor("out", [POS * BLK, D_OUT], f32, kind="ExternalOutput")

    with tile.TileContext(nc) as tc, ExitStack() as ctx:
        const = ctx.enter_context(tc.tile_pool(name="const", bufs=1))
        xg_pool = ctx.enter_context(tc.tile_pool(name="xg", bufs=12))
        oh_pool = ctx.enter_context(tc.tile_pool(name="oh", bufs=8))
        sts_pool = ctx.enter_context(tc.tile_pool(name="sts", bufs=6))
        outs_pool = ctx.enter_context(tc.tile_pool(name="outs", bufs=6))
        st_psum = ctx.enter_context(tc.tile_pool(name="stp", bufs=6, space="PSUM"))
        o_psum = ctx.enter_context(tc.tile_pool(name="op", bufs=2, space="PSUM"))

        w_t = const.tile([D_IN, D_OUT], f32)
        nc.sync.dma_start(w_t[:], w_d[:])
        bias_t = const.tile([128, D_OUT], f32)
        nc.sync.dma_start(bias_t[:], bias_d[:])
        iota_t = const.tile([128, BLK], f32)
        nc.sync.dma_start(iota_t[:], iota_d[:])
        idx_t = const.tile([128, ntot // 16], mybir.dt.int16)
        nc.sync.dma_start(idx_t[:], idx_d[:])
        dstm_t = const.tile([128, ttot], f32)
        nc.sync.dma_start(dstm_t[:], dstm_d[:])
        ewm_t = const.tile([128, ttot], f32)
        nc.sync.dma_start(ewm_t[:], ewm_d[:])
        wsum_t = const.tile([128, POS], f32)
        nc.sync.dma_start(wsum_t[:], wsum_d[:])

        for hg in range(N_HG if hg_limit is None else hg_limit):
            ps = range(hg * HG, hg * HG + HG)
            xg = {}
            for s in range(N_SBIN):
                n = int(call_n[hg, s])
                if n == 0:
                    continue
                g_t = xg_pool.tile([128, n // BLK, D_IN], f32, tag="xg")
                lo = s * SRC_CH
                hi = min(lo + SRC_CH, N_NODES)
                c0 = int(call_off[hg, s]) // 16
                if skip_gather:
                    nc.vector.memset(g_t[:, 0:1, 0:1], 0.0)
                elif True:
                    nc.gpsimd.dma_gather(
                        g_t[:], x_d[lo:hi, :], idx_t[:, c0 : c0 + n // 16],
                        n, n, D_IN, single_packet=False,
                        queue_num=(hg * N_SBIN + s) % 2,
                    )
                xg[s] = g_t

            for p in ps:
                tot = int(K[p].sum())
                if skip_compute:
                    continue
                if tot == 0:
                    z_t = outs_pool.tile([128, D_OUT], f32, tag="outs")
                    nc.vector.memset(z_t[:], 0.0)
                    nc.sync.dma_start(out_d[p * BLK : (p + 1) * BLK, :], z_t[:])
                    continue
                st_p = st_psum.tile([D_IN, BLK], f32, tag="stp")
                done = 0
                for s in range(N_SBIN):
                    kps = int(K[p, s])
                    if kps == 0:
                        continue
                    # chunk column base of p inside this (hg, s) gather
                    base = sum(int(K[q, s]) for q in ps if q < p)
                    for k in range(kps):
                        t = int(eoff[p, s]) + k
                        oh_t = oh_pool.tile([128, BLK], f32, tag="oh")
                        nc.vector.tensor_scalar(
                            oh_t[:], iota_t[:],
                            dstm_t[:, t : t + 1], ewm_t[:, t : t + 1],
                            op0=mybir.AluOpType.is_equal,
                            op1=mybir.AluOpType.mult,
                        )
                        nc.tensor.matmul(
                            st_p[:], xg[s][:, base + k, :], oh_t[:],
                            start=(done == 0), stop=(done == tot - 1),
                        )
                        done += 1
                st_s = sts_pool.tile([D_IN, BLK], f32, tag="sts")
                nc.vector.tensor_copy(st_s[:], st_p[:])
                o_p = o_psum.tile([BLK, D_OUT], f32, tag="op")
                nc.tensor.matmul(o_p[:], st_s[:], w_t[:], start=True, stop=True)
                # out = S@W + wsum[node] * b   (each edge contributes w_e * b)
                bb_t = oh_pool.tile([BLK, D_OUT], f32, tag="bb")
                nc.vector.tensor_scalar(
                    bb_t[:], bias_t[:], wsum_t[:, p : p + 1], None,
                    op0=mybir.AluOpType.mult,
                )
                o_t = outs_pool.tile([BLK, D_OUT], f32, tag="outs")
                nc.vector.tensor_add(o_t[:], o_p[:], bb_t[:])
                nc.sync.dma_start(out_d[p * BLK : (p + 1) * BLK, :], o_t[:])

    nc.compile()
    return nc


def _prep_inputs(x, W, b, edge_w, src, dst, order, counts, seg_start,
                 pos_block, K, goff, eoff, ntot, ttot):
    src_s = src[order]
    dst_s = dst[order]
    ew_s = edge_w[order]

    idx_stream = np.zeros((N_CORES, ntot), np.int16)
    dst_stream = np.zeros((N_CORES, ttot * BLK), np.float32)
    ew_stream = np.zeros((N_CORES, ttot * BLK), np.float32)

    for c in range(N_CORES):
        for p in range(POS):
            blk = int(pos_block[c, p])
            if blk < 0:
                continue
            for s in range(N_SBIN):
                cnt = int(counts[blk, s])
                if cnt == 0:
                    continue
                a = int(seg_start[blk * N_SBIN + s])
                sl = slice(a, a + cnt)
                g0 = int(goff[p, s])
                idx_stream[c, g0 : g0 + cnt] = (src_s[sl] - (s << 15)).astype(
                    np.int16
                )
                e0 = int(eoff[p, s]) * BLK
                dst_stream[c, e0 : e0 + cnt] = (dst_s[sl] & (BLK - 1)).astype(
                    np.float32
                )
                ew_stream[c, e0 : e0 + cnt] = ew_s[sl]

    iota = np.tile(np.arange(BLK, dtype=np.float32), (128, 1))
    bias = np.tile(b.astype(np.float32), (128, 1))

    # per-node sum of incoming edge weights (bias scale): out += wsum * b
    wsum_full = np.zeros(N_BLOCKS * BLK, np.float32)
    np.add.at(wsum_full, dst, edge_w)
    wsum_core = np.zeros((N_CORES, POS, BLK), np.float32)
    for c in range(N_CORES):
        for p in range(POS):
            blk = int(pos_block[c, p])
            if blk >= 0:
                wsum_core[c, p] = wsum_full[blk * BLK : (blk + 1) * BLK]

    in_maps = []
    for c in range(N_CORES):
        in_maps.append({
            "x": x,
            "w": W,
            "bias": bias,
            "iota": iota,
            "idx": np.tile(
                idx_stream[c].reshape(-1, 16).T.copy(), (8, 1)
            ),
            "dstm": np.ascontiguousarray(dst_stream[c].reshape(-1, BLK).T),
            "ewm": np.ascontiguousarray(ew_stream[c].reshape(-1, BLK).T),
            "wsum": np.ascontiguousarray(wsum_core[c].T),
        })
    return in_maps


def build_and_run(inputs, **run_kwargs):
    """Returns (BassKernelResults, assemble_fn). assemble_fn(results)->out."""
    x = np.ascontiguousarray(np.asarray(inputs["x"], dtype=np.float32))
    W = np.ascontiguousarray(np.asarray(inputs["W"], dtype=np.float32))
    b = np.asarray(inputs["b"], dtype=np.float32)
    edge_w = np.asarray(inputs["edge_w"], dtype=np.float32)
    src = np.asarray(inputs["src"]).astype(np.int64)
    dst = np.asarray(inputs["dst"]).astype(np.int64)

    order, counts, seg_start, pos_block, segcnt, K = _plan(src, dst)
    goff, call_off, call_n, eoff, ntot, ttot = _offsets(K)
    in_maps = _prep_inputs(x, W, b, edge_w, src, dst, order, counts, seg_start,
                           pos_block, K, goff, eoff, ntot, ttot)
    nc = _build(K, call_off, call_n, eoff, ntot, ttot)
    res = run_bass_kernel_spmd(nc, in_maps, core_ids=list(range(N_CORES)),
                               **run_kwargs)

    def assemble(results):
        out = np.zeros((N_NODES, D_OUT), np.float32)
        for c in range(N_CORES):
            oc = results[c]["out"]
            for p in range(POS):
                blk = int(pos_block[c, p])
                if blk < 0:
                    continue
                lo = blk * BLK
                hi = min(lo + BLK, N_NODES)
                out[lo:hi] = oc[p * BLK : p * BLK + (hi - lo)]
        return out

    return res, assemble


def kernel(**inputs) -> np.ndarray:
    res, assemble = build_and_run(inputs)
    return assemble(res.results)



# revision 4
# speedup vs baseline: 1.2195x; 1.2195x over previous
"""GCNConv (gather -> scale -> segment-sum -> linear) on 8 trn2 NeuronCores.

Math: out = segment_sum(h[src] * edge_w, dst), h = x @ W + b.
Per 128-node destination block B:
    out[B] = (sum_e onehot(dst_e)^T (edge_w_e * x[src_e])) @ W + wsum*b
with ST[din, dst] += xg_chunk^T @ oh_chunk accumulated in PSUM per block.

v3 (measured-cost driven):
  - one synchronous dma_gather per (pos, sbin) segment, round-robin over
    4 SWDGE queues: measured 3.1-4.8 ns/descriptor vs 5.1 at 2 queues
    and ~10 in v1's (halfgroup, sbin) batching. Sync mode keeps Tile's
    dependency tracking sound (the instruction's sem is the DMA sem).
  - gathered fp32 x tiles are cast to bf16 on the otherwise-idle
    Activation engine (one activation per gather call); the one-hot is
    built in bf16 on DVE (measured 230 ns/op; fp32 is 263); chunk
    matmuls run as bf16 (measured 64 ns vs 240 fp32 per LDW+MM).
  - ST tiles are copy-cast PSUM->SBUF(bf16) on ACT; projection matmul
    (ST^T @ W) in bf16.

Sharding: destination-owner, 782 dst blocks balanced over 8 cores by
edge count; indices for dma_gather are int16 so x is viewed as 4
sub-tables of <=32768 rows and edges are binned by (dst block, src>>15).
All loop bounds are max-over-cores and baked into one SPMD program.
"""

import sys

sys.path.insert(0, "/opt/trn_rl_repo")

from contextlib import ExitStack

import numpy as np

import concourse.bacc as bacc
import concourse.mybir as mybir
from concourse import tile
from concourse.bass_utils import run_bass_kernel_spmd

N_NODES = 100000
N_EDGES = 1600000
D_IN = 128
D_OUT = 64
N_CORES = 8
BLK = 128                      # dst nodes per block (= one-hot width)
SRC_CH = 1 << 15               # rows per gather sub-table (int16 idx limit)
N_SBIN = (N_NODES + SRC_CH - 1) // SRC_CH      # 4
N_BLOCKS = (N_NODES + BLK - 1) // BLK          # 782
POS = 99                       # positions per core
N_QUEUES = 4


def _plan(src, dst):
    """Derive the shared SPMD loop structure + per-core edge orderings."""
    block = dst >> 7
    sbin = src >> 15
    key = block * N_SBIN + sbin
    order = np.argsort(key, kind="stable")
    counts = np.bincount(key, minlength=N_BLOCKS * N_SBIN).reshape(N_BLOCKS, N_SBIN)
    seg_start = np.zeros(N_BLOCKS * N_SBIN + 1, np.int64)
    seg_start[1:] = np.cumsum(counts.ravel())

    # Balanced assignment of blocks to cores (greedy, largest first).
    blk_tot = counts.sum(1)
    pos_block = np.full((N_CORES, POS), -1, np.int64)
    core_tot = np.zeros(N_CORES, np.int64)
    core_n = np.zeros(N_CORES, np.int64)
    for b in np.argsort(-blk_tot, kind="stable"):
        ok = np.where(core_n < POS)[0]
        c = ok[np.argmin(core_tot[ok])]
        pos_block[c, core_n[c]] = b
        core_n[c] += 1
        core_tot[c] += blk_tot[b]
    # positions on each core sorted by size desc -> rank-matched across cores
    for c in range(N_CORES):
        nb = int(core_n[c])
        bl = pos_block[c, :nb]
        pos_block[c, :nb] = bl[np.argsort(-blk_tot[bl], kind="stable")]

    # seg sizes [core, pos, sbin]; chunks per (pos, sbin) = max over cores
    segcnt = np.where(
        pos_block[:, :, None] >= 0,
        counts[np.clip(pos_block, 0, None)],
        0,
    )                                             # [8, POS, 4]
    K = -(-segcnt.max(0) // BLK)                  # [POS, 4] chunks (ceil)
    return order, counts, seg_start, pos_block, segcnt, K


def _offsets(K):
    """Static stream layouts shared by all cores (per (pos, sbin))."""
    goff = np.zeros((POS, N_SBIN), np.int64)      # idx slot offset of (p, s)
    eoff = np.zeros((POS, N_SBIN), np.int64)      # chunk offset of (p, s)
    g = 0
    t = 0
    for p in range(POS):
        for s in range(N_SBIN):
            goff[p, s] = g
            g += int(K[p, s]) * BLK
            eoff[p, s] = t
            t += int(K[p, s])
    return goff, eoff, int(g), int(t)


def _build(K, goff, eoff, ntot, ttot):
    nc = bacc.Bacc("TRN2", target_bir_lowering=False, debug=False,
                   enable_asserts=False, num_devices=N_CORES,
                   num_swdge_queues=N_QUEUES)
    f32 = mybir.dt.float32
    bf16 = mybir.dt.bfloat16
    x_d = nc.dram_tensor("x", [N_NODES, D_IN], f32, kind="ExternalInput")
    w_d = nc.dram_tensor("w", [D_IN, D_OUT], f32, kind="ExternalInput")
    bias_d = nc.dram_tensor("bias", [128, D_OUT], f32, kind="ExternalInput")
    iota_d = nc.dram_tensor("iota", [128, BLK], bf16, kind="ExternalInput")
    idx_d = nc.dram_tensor("idx", [128, ntot // 16], mybir.dt.int16,
                           kind="ExternalInput")
    dstm_d = nc.dram_tensor("dstm", [128, ttot], f32, kind="ExternalInput")
    ewm_d = nc.dram_tensor("ewm", [128, ttot], f32, kind="ExternalInput")
    wsum_d = nc.dram_tensor("wsum", [128, POS], f32, kind="ExternalInput")
    out_d = nc.dram_tensor("out", [POS * BLK, D_OUT], f32, kind="ExternalOutput")

    with tile.TileContext(nc) as tc, ExitStack() as ctx:
        const = ctx.enter_context(tc.tile_pool(name="const", bufs=1))
        xg_pool = ctx.enter_context(tc.tile_pool(name="xg", bufs=16))
        xb_pool = ctx.enter_context(tc.tile_pool(name="xb", bufs=16))
        oh_pool = ctx.enter_context(tc.tile_pool(name="oh", bufs=8))
        sts_pool = ctx.enter_context(tc.tile_pool(name="sts", bufs=4))
        outs_pool = ctx.enter_context(tc.tile_pool(name="outs", bufs=6))
        st_psum = ctx.enter_context(tc.tile_pool(name="stp", bufs=6, space="PSUM"))
        o_psum = ctx.enter_context(tc.tile_pool(name="op", bufs=2, space="PSUM"))

        w_t = const.tile([D_IN, D_OUT], f32)
        nc.sync.dma_start(w_t[:], w_d[:])
        wb_t = const.tile([D_IN, D_OUT], bf16)
        nc.vector.tensor_copy(wb_t[:], w_t[:])
        bias_t = const.tile([128, D_OUT], f32)
        nc.sync.dma_start(bias_t[:], bias_d[:])
        iota_t = const.tile([128, BLK], bf16)
        nc.sync.dma_start(iota_t[:], iota_d[:])
        idx_t = const.tile([128, ntot // 16], mybir.dt.int16)
        nc.sync.dma_start(idx_t[:], idx_d[:])
        dstm_t = const.tile([128, ttot], f32)
        nc.sync.dma_start(dstm_t[:], dstm_d[:])
        ewm_t = const.tile([128, ttot], f32)
        nc.sync.dma_start(ewm_t[:], ewm_d[:])
        wsum_t = const.tile([128, POS], f32)
        nc.sync.dma_start(wsum_t[:], wsum_d[:])

        qi = 0
        for p in range(POS):
            tot = int(K[p].sum())
            if tot == 0:
                z_t = outs_pool.tile([128, D_OUT], f32, tag="outs")
                nc.vector.memset(z_t[:], 0.0)
                nc.sync.dma_start(out_d[p * BLK : (p + 1) * BLK, :], z_t[:])
                continue
            xb = {}
            for s in range(N_SBIN):
                kps = int(K[p, s])
                if kps == 0:
                    continue
                n = kps * BLK
                g_t = xg_pool.tile([128, kps, D_IN], f32, tag="xg")
                lo = s * SRC_CH
                hi = min(lo + SRC_CH, N_NODES)
                c0 = int(goff[p, s]) // 16
                nc.gpsimd.dma_gather(
                    g_t[:], x_d[lo:hi, :], idx_t[:, c0 : c0 + n // 16],
                    n, n, D_IN, single_packet=False, queue_num=qi % N_QUEUES,
                )
                qi += 1
                b_t = xb_pool.tile([128, kps, D_IN], bf16, tag="xb")
                nc.scalar.copy(b_t[:], g_t[:])
                xb[s] = b_t

            st_p = st_psum.tile([D_IN, BLK], f32, tag="stp")
            done = 0
            for s in range(N_SBIN):
                kps = int(K[p, s])
                for k in range(kps):
                    t = int(eoff[p, s]) + k
                    oh_t = oh_pool.tile([128, BLK], bf16, tag="oh")
                    nc.vector.tensor_scalar(
                        oh_t[:], iota_t[:],
                        dstm_t[:, t : t + 1], ewm_t[:, t : t + 1],
                        op0=mybir.AluOpType.is_equal,
                        op1=mybir.AluOpType.mult,
                    )
                    nc.tensor.matmul(
                        st_p[:], xb[s][:, k, :], oh_t[:],
                        start=(done == 0), stop=(done == tot - 1),
                    )
                    done += 1
            st_s = sts_pool.tile([D_IN, BLK], bf16, tag="sts")
            nc.scalar.copy(st_s[:], st_p[:])
            o_p = o_psum.tile([BLK, D_OUT], f32, tag="op")
            nc.tensor.matmul(o_p[:], st_s[:], wb_t[:], start=True, stop=True)
            # out = S@W + wsum[node] * b   (each edge contributes w_e * b)
            bb_t = outs_pool.tile([BLK, D_OUT], f32, tag="bb")
            nc.vector.tensor_scalar(
                bb_t[:], bias_t[:], wsum_t[:, p : p + 1], None,
                op0=mybir.AluOpType.mult,
            )
            o_t = outs_pool.tile([BLK, D_OUT], f32, tag="outs")
            nc.vector.tensor_add(o_t[:], o_p[:], bb_t[:])
            nc.sync.dma_start(out_d[p * BLK : (p + 1) * BLK, :], o_t[:])

    nc.compile()
    return nc


def _prep_inputs(x, W, b, edge_w, src, dst, order, counts, seg_start,
                 pos_block, K, goff, eoff, ntot, ttot):
    import ml_dtypes

    src_s = src[order]
    dst_s = dst[order]
    ew_s = edge_w[order]

    idx_stream = np.zeros((N_CORES, ntot), np.int16)
    dst_stream = np.zeros((N_CORES, ttot * BLK), np.float32)
    ew_stream = np.zeros((N_CORES, ttot * BLK), np.float32)

    for c in range(N_CORES):
        for p in range(POS):
            blk = int(pos_block[c, p])
            if blk < 0:
                continue
            for s in range(N_SBIN):
                cnt = int(counts[blk, s])
                if cnt == 0:
                    continue
                a = int(seg_start[blk * N_SBIN + s])
                sl = slice(a, a + cnt)
                g0 = int(goff[p, s])
                idx_stream[c, g0 : g0 + cnt] = (src_s[sl] - (s << 15)).astype(
                    np.int16
                )
                e0 = int(eoff[p, s]) * BLK
                dst_stream[c, e0 : e0 + cnt] = (dst_s[sl] & (BLK - 1)).astype(
                    np.float32
                )
                ew_stream[c, e0 : e0 + cnt] = ew_s[sl]

    iota = np.tile(np.arange(BLK, dtype=np.float32), (128, 1)).astype(
        ml_dtypes.bfloat16
    )
    bias = np.tile(b.astype(np.float32), (128, 1))

    # per-node sum of incoming edge weights (bias scale): out += wsum * b
    wsum_full = np.zeros(N_BLOCKS * BLK, np.float32)
    np.add.at(wsum_full, dst, edge_w)
    wsum_core = np.zeros((N_CORES, POS, BLK), np.float32)
    for c in range(N_CORES):
        for p in range(POS):
            blk = int(pos_block[c, p])
            if blk >= 0:
                wsum_core[c, p] = wsum_full[blk * BLK : (blk + 1) * BLK]

    in_maps = []
    for c in range(N_CORES):
        in_maps.append({
            "x": x,
            "w": W,
            "bias": bias,
            "iota": iota,
            "idx": np.tile(
                idx_stream[c].reshape(-1, 16).T.copy(), (8, 1)
            ),
            "dstm": np.ascontiguousarray(dst_stream[c].reshape(-1, BLK).T),
            "ewm": np.ascontiguousarray(ew_stream[c].reshape(-1, BLK).T),
            "wsum": np.ascontiguousarray(wsum_core[c].T),
        })
    return in_maps


def build_and_run(inputs, **run_kwargs):
    """Returns (BassKernelResults, assemble_fn). assemble_fn(results)->out."""
    x = np.ascontiguousarray(np.asarray(inputs["x"], dtype=np.float32))
    W = np.ascontiguousarray(np.asarray(inputs["W"], dtype=np.float32))
    b = np.asarray(inputs["b"], dtype=np.float32)
    edge_w = np.asarray(inputs["edge_w"], dtype=np.float32)
    src = np.asarray(inputs["src"]).astype(np.int64)
    dst = np.asarray(inputs["dst"]).astype(np.int64)

    order, counts, seg_start, pos_block, segcnt, K = _plan(src, dst)
    goff, eoff, ntot, ttot = _offsets(K)
    in_maps = _prep_inputs(x, W, b, edge_w, src, dst, order, counts, seg_start,
                           pos_block, K, goff, eoff, ntot, ttot)
    nc = _build(K, goff, eoff, ntot, ttot)
    res = run_bass_kernel_spmd(nc, in_maps, core_ids=list(range(N_CORES)),
                               **run_kwargs)

    def assemble(results):
        out = np.zeros((N_NODES, D_OUT), np.float32)
        for c in range(N_CORES):
            oc = results[c]["out"]
            for p in range(POS):
                blk = int(pos_block[c, p])
                if blk < 0:
                    continue
                lo = blk * BLK
                hi = min(lo + BLK, N_NODES)
                out[lo:hi] = oc[p * BLK : p * BLK + (hi - lo)]
        return out

    return res, assemble


def kernel(**inputs) -> np.ndarray:
    res, assemble = build_and_run(inputs)
    return assemble(res.results)
